# revision 14
# baseline (speedup 1.0000x reference)
"""Trainium2 Bass kernel for nn_AdvancedKoopmanModel (GNN message passing + Koopman scan).

8-core SPMD strategy:
- Host sorts edges by dst, shards by dst-range (core c owns nodes [c*1024,(c+1)*1024)),
  pads each 128-node window's edge list to a common multiple of 128 across cores.
- Conv layers decompose through segment_sum linearity:
    z_e = relu(U[dst_e] + V[src_e] + ea_e@W1c^T + b1);  out = segsum(z)@W2^T + deg*b2
  U,V are node-level matmuls; only V is AllGathered (bf16).  dst-broadcast and
  segment-sum are block one-hot matmuls (host-built A/A^T, bf16, SBUF-resident,
  shared by all 9 conv layers).  src-gather = gpsimd dma_gather from DRAM.
- Koopman rollout: feature-major Kogge-Stone over each core's 1024 chunk,
  cross-core carries via one tiny AllGather, K powers precomputed in fp64.
"""
import sys
sys.path.insert(0, '/opt/trn_rl_repo')
from contextlib import ExitStack

import numpy as np
import ml_dtypes

import concourse.bass as bass
import concourse.bacc as bacc
import concourse.tile as tile
import concourse.mybir as mybir
from concourse import bass_utils
from concourse.masks import make_identity

FP32 = mybir.dt.float32
BF16 = mybir.dt.bfloat16
I16 = mybir.dt.int16
AF = mybir.ActivationFunctionType
ALU = mybir.AluOpType
BF16NP = ml_dtypes.bfloat16

T, E, D_IN, D_H, KD, U, EPS = 8192, 131072, 64, 256, 256, 4, 1e-5
NCN, P = 8, 128
TL = T // NCN
NWIN = TL // P
GCHUNK = 8                      # gather chunk: 8 tiles = 1024 rows
NT512 = TL // 512               # 512-wide tiles over the local node axis

ENC_CONVS = [(D_IN, D_H), (D_H, D_H // 2), (D_H // 2, KD)]
DEC_CONVS = [(KD, D_H), (D_H, D_H // 2), (D_H // 2, D_IN)]
ENC_FF = [D_IN, D_H, D_H, D_H // 2, KD]
DEC_FF = [KD, D_H, D_H, D_H // 2, D_IN]


def _ceil(a, b):
    return -(-a // b)


def _np(t):
    return np.asarray(t)


# ---------------------------------------------------------------------------
# host preprocessing
# ---------------------------------------------------------------------------

def _fold_params(params):
    p = {k: ((_np(v[0]).astype(np.float64), _np(v[1]).astype(np.float64))
             if isinstance(v, tuple) else _np(v)) for k, v in params.items()}
    convs = []
    prev_g = prev_b = None
    for ka, kb, kn in (('c1a', 'c1b', 'n1'), ('c2a', 'c2b', 'n2'), ('c3a', 'c3b', 'n3')):
        W1, b1 = p[ka]
        W2, b2 = p[kb]
        din = (W1.shape[1] - U) // 2
        W1a, W1b, W1c = W1[:, :din], W1[:, din:2 * din], W1[:, 2 * din:]
        if prev_g is not None:
            b1 = b1 + prev_b @ W1a.T + prev_b @ W1b.T
            W1a = W1a * prev_g[None, :]
            W1b = W1b * prev_g[None, :]
        convs.append(dict(W1aT=W1a.T, W1bT=W1b.T, W1c=W1c, b1=b1, W2T=W2.T, b2=b2))
        prev_g, prev_b = p[kn]
    ff = [(p[f'f{i}'][0], p[f'f{i}'][1]) for i in range(1, 5)]
    return convs, ff, prev_g, prev_b


def _host_prep(edge_index, edge_attr):
    ei = _np(edge_index).astype(np.int64)
    ea = _np(edge_attr).astype(np.float32)
    src_all, dst_all = ei[0], ei[1]
    order = np.argsort(dst_all, kind='stable')
    cores = []
    for c in range(NCN):
        lo = c * TL
        sel = order[(dst_all[order] >= lo) & (dst_all[order] < lo + TL)]
        wins = (dst_all[sel] - lo) // P
        cores.append((lo, [sel[wins == w] for w in range(NWIN)]))
    wpad = [-(-max(len(cores[c][1][w]) for c in range(NCN)) // P) * P
            for w in range(NWIN)]
    epad = sum(wpad)
    win_of_tile = []
    for w in range(NWIN):
        win_of_tile += [w] * (wpad[w] // P)

    per_core = []
    for c in range(NCN):
        lo, by_win = cores[c]
        srcp = np.zeros(epad, np.int64)
        dstl = np.zeros(epad, np.int64)
        eap = np.zeros((epad, U), np.float32)
        valid = np.zeros(epad, np.float32)
        off = 0
        for w in range(NWIN):
            e = by_win[w]
            srcp[off:off + len(e)] = src_all[e]
            dstl[off:off + len(e)] = dst_all[e] - lo - w * P
            eap[off:off + len(e)] = ea[e]
            valid[off:off + len(e)] = 1.0
            off += wpad[w]
        ar = np.arange(epad)
        AT = np.zeros((P, epad), np.float32)
        AT[dstl, ar] = valid
        A = np.zeros((P, epad), np.float32)
        A[ar % P, (ar // P) * P + dstl] = valid
        winv = np.repeat(np.arange(NWIN), np.array(wpad))
        deg = np.zeros(TL, np.float32)
        np.add.at(deg, winv * P + dstl, valid)
        w16 = np.zeros((16, epad // 16), np.int16)
        w16[ar % 16, ar // 16] = srcp.astype(np.int16)
        idx = np.tile(w16, (8, 1))
        eaT1 = np.concatenate([eap.T, np.ones((1, epad), np.float32)], 0).astype(BF16NP)
        per_core.append(dict(AT=AT.astype(BF16NP), A=A.astype(BF16NP), deg=deg,
                             idx=idx, eaT1=eaT1))
    return per_core, (epad // P, epad, tuple(win_of_tile))


def _weight_arrays(convs, ff, g3, b3, pfx):
    arrs = {}
    for i, L in enumerate(convs):
        arrs[f'{pfx}w1ab{i}'] = np.concatenate(
            [L['W1aT'], L['W1bT']], 1).astype(np.float32)
        arrs[f'{pfx}w1cb{i}'] = np.concatenate(
            [L['W1c'].T, L['b1'][None, :]], 0).astype(np.float32)
        arrs[f'{pfx}w2t{i}'] = L['W2T'].astype(np.float32)
        arrs[f'{pfx}b2{i}'] = L['b2'][None, :].astype(np.float32)
    for i, (Wf, bf) in enumerate(ff):
        sc = 0.5 if i == 3 else 1.0
        arrs[f'{pfx}ffw{i}'] = (Wf.T * sc).astype(np.float32)
        bb = bf * sc + (0.5 * b3 if i == 3 else 0.0)
        arrs[f'{pfx}ffb{i}'] = bb[:, None].astype(np.float32)
        if i == 3:
            arrs[f'{pfx}ffb3r'] = bb[None, :].astype(np.float32)
    arrs[f'{pfx}g3h'] = (0.5 * g3)[:, None].astype(np.float32)
    return arrs


# ---------------------------------------------------------------------------
# device kernel
# ---------------------------------------------------------------------------

def build_kernel(meta):
    ntiles, epad, wot = meta

    nc = bacc.Bacc("TRN2", target_bir_lowering=False, debug=False,
                   enable_asserts=True, num_devices=NCN)
    din_ = {}

    def dram_in(name, shape, dt=FP32):
        din_[name] = nc.dram_tensor(name, list(shape), dt, kind="ExternalInput")

    for pfx, convs, ffd in (('e_', ENC_CONVS, ENC_FF), ('d_', DEC_CONVS, DEC_FF)):
        go = KD if pfx == 'e_' else D_IN
        for i, (dn, dm) in enumerate(convs):
            dmo = convs[i + 1][0] if i < 2 else go
            dram_in(f'{pfx}w1ab{i}', [dn, 2 * dm])
            dram_in(f'{pfx}w1cb{i}', [5, dm])
            dram_in(f'{pfx}w2t{i}', [dm, dmo])
            dram_in(f'{pfx}b2{i}', [1, dmo])
        for i in range(4):
            dram_in(f'{pfx}ffw{i}', [ffd[i], ffd[i + 1]])
            dram_in(f'{pfx}ffb{i}', [ffd[i + 1], 1])
        dram_in(f'{pfx}ffb3r', [1, ffd[4]])
        dram_in(f'{pfx}g3h', [go, 1])
    dram_in('kp', [11, KD, KD])
    dram_in('lwt', [U, KD])
    dram_in('at_oh', [P, epad], BF16)
    dram_in('a_oh', [P, epad], BF16)
    dram_in('idx', [P, epad // 16], I16)
    dram_in('eat1', [5, epad], BF16)
    dram_in('deg', [1, TL])
    dram_in('xt', [D_IN, TL])
    dram_in('east', [U, TL])
    dram_in('sel', [P, NCN])
    out_ks = nc.dram_tensor('ks_t', [KD, TL], FP32, kind="ExternalOutput")
    out_ae = nc.dram_tensor('ae_t', [D_IN, TL], FP32, kind="ExternalOutput")
    out_ro = nc.dram_tensor('ro_t', [D_IN, TL], FP32, kind="ExternalOutput")

    with tile.TileContext(nc) as tc, ExitStack() as ctx:
        cpool = ctx.enter_context(tc.tile_pool(name="const", bufs=1))
        dram = ctx.enter_context(tc.tile_pool(name="dram", bufs=2, space="DRAM"))
        wpool = ctx.enter_context(tc.tile_pool(name="wts", bufs=2))
        hpool = ctx.enter_context(tc.tile_pool(name="hbuf", bufs=1))
        ffpool = ctx.enter_context(tc.tile_pool(name="ffb", bufs=2))
        vepool = ctx.enter_context(tc.tile_pool(name="ve", bufs=2))
        zpool = ctx.enter_context(tc.tile_pool(name="z", bufs=2))
        eapool = ctx.enter_context(tc.tile_pool(name="ea", bufs=2))
        scrpool = ctx.enter_context(tc.tile_pool(name="scr", bufs=1))
        ps_z = ctx.enter_context(tc.tile_pool(name="ps_z", bufs=2, space="PSUM"))
        ps_s = ctx.enter_context(tc.tile_pool(name="ps_s", bufs=2, space="PSUM"))
        ps_nm = ctx.enter_context(tc.tile_pool(name="ps_nm", bufs=4, space="PSUM"))

        ident = cpool.tile([P, P], FP32)
        make_identity(nc, ident[:])
        ident_bf = cpool.tile([P, P], BF16)
        nc.vector.tensor_copy(ident_bf[:], ident[:])
        at_sb = cpool.tile([P, epad], BF16)
        nc.sync.dma_start(at_sb[:], din_['at_oh'][:, :])
        a_sb = cpool.tile([P, epad], BF16)
        nc.sync.dma_start(a_sb[:], din_['a_oh'][:, :])
        idx_sb = cpool.tile([P, epad // 16], I16)
        nc.sync.dma_start(idx_sb[:], din_['idx'][:, :])
        deg_sb = cpool.tile([1, TL], FP32)
        nc.sync.dma_start(deg_sb[:], din_['deg'][:, :])
        ones_sb = cpool.tile([1, 512], FP32)
        nc.vector.memset(ones_sb[:], 1.0)
        sel_sb = cpool.tile([P, NCN], FP32)
        nc.sync.dma_start(sel_sb[:], din_['sel'][:, :])
        xt_sb = cpool.tile([D_IN, TL], FP32)
        nc.sync.dma_start(xt_sb[:], din_['xt'][:, :])
        east_sb = cpool.tile([U, TL], FP32)
        nc.sync.dma_start(east_sb[:], din_['east'][:, :])
        lwt_sb = cpool.tile([U, KD], FP32)
        nc.sync.dma_start(lwt_sb[:], din_['lwt'][:, :])

        def load_chunked(name, rows, cols, tag, dt=FP32):
            kc = _ceil(rows, P)
            t = wpool.tile([P, kc, cols], dt, tag=tag)
            eng = nc.sync if dt == FP32 else nc.gpsimd
            if kc == 1:
                eng.dma_start(t[:rows, 0, :], din_[name][:, :])
            else:
                eng.dma_start(
                    t[:], din_[name][:, :].rearrange("(k p) c -> p k c", p=P))
            return t

        def load_kp(level):
            t = wpool.tile([P, 2, KD], FP32, tag="kp")
            nc.sync.dma_start(
                t[:], din_['kp'][level, :, :].rearrange("(k p) c -> p k c", p=P))
            return t

        # ------------------------------------------------------------------
        def gnn(pfx, convs, ffd, hT0, out_dram):
            """hT0: list of per-128-chunk feature-major APs [kk, TL].
            Writes GNN output (feature-major) to out_dram; returns the output
            SBUF tile (kept in hpool tag 'gout_'+pfx-ish) and its chunk list."""
            go = KD if pfx == 'e_' else D_IN
            gcn = _ceil(go, P)
            cur = hT0

            # ---- ff layers 1..3 (feature-major), emitted first to overlap AGs
            fcur, fdims = hT0, ffd[0]
            for fi in range(3):
                dni, dno = ffd[fi], ffd[fi + 1]
                fw = load_chunked(f'{pfx}ffw{fi}', dni, dno, "ffw")
                fb = wpool.tile([P, _ceil(dno, P)], FP32, tag="ffbias")
                nc.sync.dma_start(fb[:], din_[f'{pfx}ffb{fi}'][:, :]
                                  .rearrange("(m p) o -> p (m o)", p=P))
                fnx = ffpool.tile([P, _ceil(dno, P), TL], FP32, tag="ff")
                for mc in range(_ceil(dno, P)):
                    mm = min(P, dno - mc * P)
                    for nt in range(NT512):
                        pf = ps_nm.tile([P, 512], FP32, space="PSUM", tag="nm")
                        kcn_f = _ceil(dni, P)
                        for kc in range(kcn_f):
                            kk = min(P, dni - kc * P)
                            rhs = (fcur[kc][:kk, nt * 512:(nt + 1) * 512] if fi == 0
                                   else fcur[:kk, kc, nt * 512:(nt + 1) * 512])
                            nc.tensor.matmul(pf[:mm, :], fw[:kk, kc, mc * P:mc * P + mm],
                                             rhs, start=(kc == 0), stop=(kc == kcn_f - 1))
                        nc.scalar.activation(fnx[:mm, mc, nt * 512:(nt + 1) * 512],
                                             pf[:mm, :], AF.Relu,
                                             bias=fb[:mm, mc:mc + 1])
                fcur = fnx

            # ---- conv layers
            for li, (dn, dm) in enumerate(convs):
                dmo = convs[li + 1][0] if li < 2 else go
                kcn = _ceil(dn, P)
                kdn = _ceil(dm, P)
                dm_pad = max(dm, P)
                w1ab = load_chunked(f'{pfx}w1ab{li}', dn, 2 * dm, "w1ab")
                w1cb = wpool.tile([5, dm], BF16, tag="w1cb")
                nc.gpsimd.dma_start(w1cb[:], din_[f'{pfx}w1cb{li}'][:, :])
                w2t = load_chunked(f'{pfx}w2t{li}', dm, dmo, "w2t", dt=BF16)
                b2 = wpool.tile([1, dmo], FP32, tag="b2")
                nc.sync.dma_start(b2[:], din_[f'{pfx}b2{li}'][:, :])

                ag_in = dram.tile([TL, dm_pad], BF16, tag="agin")
                ag_out = dram.tile([T, dm_pad], BF16, tag="agout")
                v_sb = hpool.tile([P, NWIN, dm_pad], BF16, tag="v_sb")
                u_sb = hpool.tile([P, NWIN, dm], BF16, tag="u_sb")
                if dm_pad > dm:
                    nc.vector.memset(v_sb[:], 0.0)
                for g in range(NWIN):
                    pv = ps_nm.tile([P, dm], FP32, space="PSUM", tag="nm")
                    for kc in range(kcn):
                        kk = min(P, dn - kc * P)
                        nc.tensor.matmul(pv[:], cur[kc][:kk, g * P:(g + 1) * P],
                                         w1ab[:kk, kc, dm:2 * dm],
                                         start=(kc == 0), stop=(kc == kcn - 1))
                    nc.scalar.activation(v_sb[:, g, :dm], pv[:], AF.Copy)
                nc.sync.dma_start(ag_in[:].rearrange("(g p) d -> p g d", p=P),
                                  v_sb[:])
                nc.gpsimd.collective_compute(
                    "AllGather", ALU.bypass, replica_groups=[list(range(NCN))],
                    ins=[ag_in.opt()], outs=[ag_out.opt()])
                for g in range(NWIN):
                    pu = ps_nm.tile([P, dm], FP32, space="PSUM", tag="nm")
                    for kc in range(kcn):
                        kk = min(P, dn - kc * P)
                        nc.tensor.matmul(pu[:], cur[kc][:kk, g * P:(g + 1) * P],
                                         w1ab[:kk, kc, 0:dm],
                                         start=(kc == 0), stop=(kc == kcn - 1))
                    nc.scalar.activation(u_sb[:, g, :], pu[:], AF.Copy)

                # ---- edge pipeline
                eac = None
                ve = None
                ps_win = None
                s_sb = hpool.tile([P, NWIN, dm], BF16, tag="s_sb")
                for t in range(ntiles):
                    w = wot[t]
                    ci, co = t // GCHUNK, t % GCHUNK
                    if co == 0:
                        base = ci * GCHUNK * P
                        ng = min(GCHUNK * P, epad - base)
                        ve = vepool.tile([P, GCHUNK, dm_pad], BF16, tag="ve")
                        nc.gpsimd.dma_gather(
                            out_ap=ve[:, :ng // P, :], in_ap=ag_out[:],
                            idxs_ap=idx_sb[:, base // 16:(base + ng) // 16],
                            num_idxs=ng, num_idxs_reg=ng, elem_size=dm_pad)
                        eac = eapool.tile([5, GCHUNK * P], BF16, tag="eac")
                        nc.sync.dma_start(eac[:, :ng],
                                          din_['eat1'][:, base:base + ng])
                    pz = ps_z.tile([P, dm], FP32, space="PSUM", tag="pz")
                    nc.tensor.matmul(pz[:], at_sb[:, t * P:(t + 1) * P],
                                     u_sb[:, w, :], start=True, stop=False)
                    nc.tensor.matmul(pz[:], eac[:, co * P:(co + 1) * P], w1cb[:, :],
                                     start=False, stop=True, skip_group_check=True)
                    nc.vector.scalar_tensor_tensor(
                        out=pz[:], in0=pz[:], scalar=1.0, op0=ALU.mult,
                        in1=ve[:, co, :dm], op1=ALU.add)
                    zt = zpool.tile([P, dm], BF16, tag="zt")
                    nc.scalar.activation(zt[:], pz[:], AF.Relu)
                    first = (t == 0) or (wot[t - 1] != w)
                    last = (t == ntiles - 1) or (wot[t + 1] != w)
                    if first:
                        ps_win = ps_s.tile([P, dm], FP32, space="PSUM", tag="psw")
                    nc.tensor.matmul(ps_win[:], a_sb[:, t * P:(t + 1) * P], zt[:],
                                     start=first, stop=last, skip_group_check=True)
                    if last:
                        nc.vector.tensor_copy(s_sb[:, w, :], ps_win[:])

                # ---- S^T
                st_sb = hpool.tile([P, kdn, TL], BF16, tag="st_sb")
                for g in range(NWIN):
                    for kc in range(kdn):
                        jj = min(P, dm - kc * P)
                        ptr = ps_nm.tile([P, P], BF16, space="PSUM", tag="nm")
                        nc.tensor.transpose(ptr[:jj, :],
                                            s_sb[:, g, kc * P:kc * P + jj],
                                            ident_bf[:])
                        nc.scalar.activation(st_sb[:jj, kc, g * P:(g + 1) * P],
                                             ptr[:jj, :], AF.Copy)

                # ---- node out + LN core (in place on r_sb)
                r_sb = hpool.tile([P, NWIN, dmo], FP32, tag="r_sb")
                sums = hpool.tile([P, NWIN, 2], FP32, tag="lnsums")
                for g in range(NWIN):
                    po = ps_nm.tile([P, dmo], FP32, space="PSUM", tag="nm")
                    for kc in range(kdn):
                        kk = min(P, dm - kc * P)
                        nc.tensor.matmul(po[:], st_sb[:kk, kc, g * P:(g + 1) * P],
                                         w2t[:kk, kc, :], start=(kc == 0), stop=False)
                    nc.tensor.matmul(po[:], deg_sb[:, g * P:(g + 1) * P], b2[:, :],
                                     start=False, stop=True, skip_group_check=True)
                    nc.scalar.activation(r_sb[:, g, :], po[:], AF.Relu,
                                         accum_out=sums[:, g, 0:1])
                    scr = scrpool.tile([P, dmo], FP32, tag="lnscr")
                    nc.scalar.activation(scr[:], r_sb[:, g, :], AF.Square,
                                         accum_out=sums[:, g, 1:2])
                st4 = hpool.tile([P, NWIN, 3], FP32, tag="lnstat")
                nc.vector.tensor_scalar(out=st4[:, :, 0], in0=sums[:, :, 0],
                                        scalar1=1.0 / dmo, scalar2=None, op0=ALU.mult)
                nc.vector.tensor_scalar(out=st4[:, :, 1], in0=sums[:, :, 1],
                                        scalar1=1.0 / dmo, scalar2=EPS,
                                        op0=ALU.mult, op1=ALU.add)
                nc.vector.tensor_tensor(out=st4[:, :, 2], in0=st4[:, :, 0],
                                        in1=st4[:, :, 0], op=ALU.mult)
                nc.vector.tensor_tensor(out=st4[:, :, 1], in0=st4[:, :, 1],
                                        in1=st4[:, :, 2], op=ALU.subtract)
                nc.scalar.activation(st4[:, :, 1], st4[:, :, 1], AF.Sqrt)
                nc.vector.reciprocal(st4[:, :, 1], st4[:, :, 1])
                nc.vector.scalar_tensor_tensor(out=st4[:, :, 2], in0=st4[:, :, 0],
                                               scalar=-1.0, op0=ALU.mult,
                                               in1=st4[:, :, 1], op1=ALU.mult)
                for g in range(NWIN):
                    nc.vector.tensor_scalar(out=r_sb[:, g, :], in0=r_sb[:, g, :],
                                            scalar1=st4[:, g, 1:2],
                                            scalar2=st4[:, g, 2:3],
                                            op0=ALU.mult, op1=ALU.add)
                # ---- transpose LN-core -> feature-major for next layer
                nxt = hpool.tile([P, _ceil(dmo, P), TL], FP32,
                                 tag=("houtT" if li == 2 else "hT"))
                for g in range(NWIN):
                    for kc in range(_ceil(dmo, P)):
                        jj = min(P, dmo - kc * P)
                        ptr = ps_nm.tile([P, P], FP32, space="PSUM", tag="nm")
                        nc.tensor.transpose(ptr[:jj, :],
                                            r_sb[:, g, kc * P:kc * P + jj], ident[:])
                        nc.scalar.activation(nxt[:jj, kc, g * P:(g + 1) * P],
                                             ptr[:jj, :], AF.Copy)
                cur = [nxt[:, kc, :] for kc in range(_ceil(dmo, P))]

            # ---- ff layer 4 (no relu) + combine with n3 affine -> output
            g3h = wpool.tile([P, gcn], FP32, tag="g3h")
            if go > P:
                nc.sync.dma_start(g3h[:], din_[f'{pfx}g3h'][:, :]
                                  .rearrange("(m p) o -> p (m o)", p=P))
            else:
                nc.sync.dma_start(g3h[:go, :], din_[f'{pfx}g3h'][:, :])
            fb3r = wpool.tile([1, go], FP32, tag="fb3r")
            nc.sync.dma_start(fb3r[:], din_[f'{pfx}ffb3r'][:, :])
            fw3 = load_chunked(f'{pfx}ffw3', ffd[3], ffd[4], "ffw3")
            outT = hpool.tile([P, gcn, TL], FP32, tag="goutT" + pfx)
            kcn_f = _ceil(ffd[3], P)
            for mc in range(gcn):
                mm = min(P, go - mc * P)
                for nt in range(NT512):
                    pf = ps_nm.tile([P, 512], FP32, space="PSUM", tag="nm")
                    for kc in range(kcn_f):
                        kk = min(P, ffd[3] - kc * P)
                        nc.tensor.matmul(pf[:mm, :], fw3[:kk, kc, mc * P:mc * P + mm],
                                         fcur[:kk, kc, nt * 512:(nt + 1) * 512],
                                         start=(kc == 0), stop=False)
                    nc.tensor.matmul(pf[:mm, :], fb3r[:, mc * P:mc * P + mm],
                                     ones_sb[:, :], start=False, stop=True,
                                     skip_group_check=True)
                    nc.vector.scalar_tensor_tensor(
                        out=outT[:mm, mc, nt * 512:(nt + 1) * 512],
                        in0=cur[mc][:mm, nt * 512:(nt + 1) * 512],
                        scalar=g3h[:mm, mc:mc + 1], op0=ALU.mult,
                        in1=pf[:mm, :], op1=ALU.add)
            if gcn > 1:
                nc.sync.dma_start(
                    out_dram[:, :].rearrange("(m p) t -> p m t", p=P), outT[:])
            else:
                nc.sync.dma_start(out_dram[:, :], outT[:go, 0, :])
            return outT, gcn

        # ---- encoder
        ksT, _ = gnn('e_', ENC_CONVS, ENC_FF, [xt_sb[:, :]], out_ks)
        ks_chunks = [ksT[:, 0, :], ksT[:, 1, :]]

        # ---- decoder on koopman states
        gnn('d_', DEC_CONVS, DEC_FF, ks_chunks, out_ae)

        # ---- koopman scan ------------------------------------------------
        bT = hpool.tile([P, 2, TL], FP32, tag="bT")
        lul = hpool.tile([P, 2, 1], FP32, tag="lul")
        nc.vector.memset(bT[:, :, 0:1], 0.0)
        for mc in range(2):
            for nt in range(NT512):
                pl = ps_nm.tile([P, 512], FP32, space="PSUM", tag="nm")
                nc.tensor.matmul(pl[:], lwt_sb[:, mc * P:(mc + 1) * P],
                                 east_sb[:, nt * 512:(nt + 1) * 512],
                                 start=True, stop=True)
                if nt == 0:
                    nc.scalar.activation(bT[:, mc, 1:513], pl[:], AF.Copy)
                else:
                    nc.scalar.activation(bT[:, mc, 513:1024], pl[:, :511], AF.Copy)
                    nc.scalar.activation(lul[:, mc, :], pl[:, 511:512], AF.Copy)
        for lvl in range(10):
            s = 1 << lvl
            kpt = load_kp(lvl)
            pks = []
            for mc in range(2):
                for base in range(0, TL - s, 512):
                    nn = min(512, TL - s - base)
                    pk = ps_nm.tile([P, 512], FP32, space="PSUM", tag="nm")
                    for kc in range(2):
                        nc.tensor.matmul(pk[:, :nn],
                                         kpt[:, kc, mc * P:(mc + 1) * P],
                                         bT[:, kc, base:base + nn],
                                         start=(kc == 0), stop=(kc == 1))
                    pks.append((mc, base, nn, pk))
            for mc, base, nn, pk in pks:
                nc.vector.tensor_tensor(out=bT[:, mc, s + base:s + base + nn],
                                        in0=bT[:, mc, s + base:s + base + nn],
                                        in1=pk[:, :nn], op=ALU.add)
        # chunk totals + rank-0 g0 -> AllGather
        agw = hpool.tile([P, 2, 2], FP32, tag="agw")
        kpt0 = load_kp(0)
        for mc in range(2):
            pt = ps_nm.tile([P, 1], FP32, space="PSUM", tag="nm")
            for kc in range(2):
                nc.tensor.matmul(pt[:], kpt0[:, kc, mc * P:(mc + 1) * P],
                                 bT[:, kc, TL - 1:TL], start=(kc == 0), stop=(kc == 1))
            nc.vector.tensor_tensor(out=agw[:, mc, 0:1], in0=pt[:],
                                    in1=lul[:, mc, :], op=ALU.add)
            nc.vector.tensor_copy(agw[:, mc, 1:2], ksT[:, mc, 0:1])
        ag2_in = dram.tile([2, KD], FP32, tag="ag2in")
        ag2_out = dram.tile([2 * NCN, KD], FP32, tag="ag2out")
        for r in range(2):
            nc.sync.dma_start(ag2_in[r, :].rearrange("(c p) -> p c", p=P),
                              agw[:, :, r])
        nc.gpsimd.collective_compute(
            "AllGather", ALU.bypass, replica_groups=[list(range(NCN))],
            ins=[ag2_in.opt()], outs=[ag2_out.opt()])
        agr = hpool.tile([2 * NCN, KD], FP32, tag="agr")
        nc.sync.dma_start(agr[:], ag2_out[:, :])
        agT = hpool.tile([P, 2, 2 * NCN], FP32, tag="agT")
        for mc in range(2):
            ptr = ps_nm.tile([P, P], FP32, space="PSUM", tag="nm")
            nc.tensor.transpose(ptr[:, :2 * NCN], agr[:, mc * P:(mc + 1) * P],
                                ident[:2 * NCN, :2 * NCN])
            nc.scalar.activation(agT[:, mc, :], ptr[:, :2 * NCN], AF.Copy)
        # carries: car[:,:,i]; car0 = g0 (rank0 row1)
        car = hpool.tile([P, 2, NCN], FP32, tag="car")
        nc.vector.tensor_copy(car[:, :, 0:1], agT[:, :, 1:2])
        kpt10 = load_kp(10)
        for i in range(NCN - 1):
            for mc in range(2):
                pc = ps_nm.tile([P, 1], FP32, space="PSUM", tag="nm")
                for kc in range(2):
                    nc.tensor.matmul(pc[:], kpt10[:, kc, mc * P:(mc + 1) * P],
                                     car[:, kc, i:i + 1], start=(kc == 0),
                                     stop=(kc == 1))
                nc.vector.tensor_tensor(out=car[:, mc, i + 1:i + 2], in0=pc[:],
                                        in1=agT[:, mc, 2 * i:2 * i + 1], op=ALU.add)
        # select own carry, write R[0]
        RT = ffpool.tile([P, 2, TL], FP32, tag="ff")
        seltmp = hpool.tile([P, NCN], FP32, tag="seltmp")
        for mc in range(2):
            nc.vector.tensor_tensor(out=seltmp[:], in0=car[:, mc, :],
                                    in1=sel_sb[:, :], op=ALU.mult)
            nc.vector.tensor_reduce(out=RT[:, mc, 0:1], in_=seltmp[:],
                                    axis=mybir.AxisListType.X, op=ALU.add)
        # doubling: R[m:2m] = R[0:m] @ K^m
        m = 1
        for lvl in range(10):
            kpt = load_kp(lvl)
            for mc in range(2):
                pd = ps_nm.tile([P, 512], FP32, space="PSUM", tag="nm")
                for kc in range(2):
                    nc.tensor.matmul(pd[:, :m], kpt[:, kc, mc * P:(mc + 1) * P],
                                     RT[:, kc, 0:m], start=(kc == 0), stop=(kc == 1))
                nc.scalar.activation(RT[:, mc, m:2 * m], pd[:, :m], AF.Copy)
            m *= 2
        for mc in range(2):
            nc.vector.tensor_tensor(out=bT[:, mc, :], in0=bT[:, mc, :],
                                    in1=RT[:, mc, :], op=ALU.add)

        # ---- decoder on rollout
        gnn('d_', DEC_CONVS, DEC_FF, [bT[:, 0, :], bT[:, 1, :]], out_ro)

    nc.compile()
    return nc


# ---------------------------------------------------------------------------
# entry point
# ---------------------------------------------------------------------------

_CACHE = {}


def kernel(x, edge_index, edge_attr, enc_params, dec_params, koopman_blocks,
           L_w, sigma):
    x = _np(x).astype(np.float32)
    ea = _np(edge_attr).astype(np.float32)
    per_core, meta = _host_prep(edge_index, edge_attr)

    e_convs, e_ff, e_g3, e_b3 = _fold_params(enc_params)
    d_convs, d_ff, d_g3, d_b3 = _fold_params(dec_params)
    shared = {}
    shared.update(_weight_arrays(e_convs, e_ff, e_g3, e_b3, 'e_'))
    shared.update(_weight_arrays(d_convs, d_ff, d_g3, d_b3, 'd_'))
    sig = _np(sigma).astype(np.float64)
    blk = _np(koopman_blocks).astype(np.float64)
    K = np.einsum('ijh,hab->ijab', sig, blk).transpose(0, 2, 1, 3).reshape(KD, KD)
    Kp = [K]
    for _ in range(10):
        Kp.append(Kp[-1] @ Kp[-1])
    shared['kp'] = np.stack([k.astype(np.float32) for k in Kp])
    shared['lwt'] = _np(L_w).astype(np.float32).T.copy()

    in_maps = []
    for c in range(NCN):
        pc = per_core[c]
        m = dict(shared)
        m['at_oh'] = pc['AT']
        m['a_oh'] = pc['A']
        m['idx'] = pc['idx']
        m['eat1'] = pc['eaT1']
        m['deg'] = pc['deg'][None, :]
        m['xt'] = np.ascontiguousarray(x[c * TL:(c + 1) * TL].T)
        m['east'] = np.ascontiguousarray(ea[c * TL:c * TL + TL].T)
        sel = np.zeros((P, NCN), np.float32)
        sel[:, c] = 1.0
        m['sel'] = sel
        in_maps.append(m)

    if meta not in _CACHE:
        _CACHE[meta] = build_kernel(meta)
    nc = _CACHE[meta]
    res = bass_utils.run_bass_kernel_spmd(nc, in_maps, core_ids=list(range(NCN)))
    ks = np.concatenate([res.results[c]['ks_t'].T for c in range(NCN)], 0)
    ae = np.concatenate([res.results[c]['ae_t'].T for c in range(NCN)], 0)
    ro = np.concatenate([res.results[c]['ro_t'].T for c in range(NCN)], 0)
    return ae.astype(np.float32), ro.astype(np.float32), ks.astype(np.float32)
